# revision 3
# baseline (speedup 1.0000x reference)
"""PixelBarlowTwins loss on 8 trn2 cores.

Weighted reformulation: with w[b,pix] = multiplicity of pix in flat_idx[b],
C_raw = sum_b Z_b diag(w_b) Zp_b^T, and all mean/var stats are w-weighted
sums -- no device gather. Each core streams 2 batches (z+zp, bf16) through
a 5-engine pipeline: DVE forms rhs=[w*zp | w], PE accumulates C_raw halves
+ s1 in PSUM, ScalarE squares (w z^2, w zp^2), Pool accumulates stats.
Host does the tiny f64 epilogue.
"""
import numpy as np
import ml_dtypes

BF = ml_dtypes.bfloat16
B, D, P, M = 16, 256, 16384, 4096
N = B * M
NCORES, BPC, CH, Q, SC = 8, 2, 128, 16, 8
EPS, LAM = 1e-6, 0.005

_CACHE = {}


def build_nc(iters):
    from concourse import bass, mybir

    F32, BF16 = mybir.dt.float32, mybir.dt.bfloat16
    Square = mybir.ActivationFunctionType.Square
    add = mybir.AluOpType.add
    K = iters
    nc = bass.Bass(target_bir_lowering=False)
    zin = nc.dram_tensor("zin", [16, 128, 4096], BF16, kind="ExternalInput")
    zpin = nc.dram_tensor("zpin", [16, 128, 4096], BF16, kind="ExternalInput")
    wsin = nc.dram_tensor("wsin", [128, 256], F32, kind="ExternalInput")
    swin = nc.dram_tensor("swin", [128, 256], F32, kind="ExternalInput")
    cps = nc.dram_tensor("cps", [4, 128, 257], F32, kind="ExternalOutput")
    accs = nc.dram_tensor("accs", [6, 128, 256], F32, kind="ExternalOutput")
    zv, zpv, cpv, acv = zin[:], zpin[:], cps[:], accs[:]

    from contextlib import ExitStack

    es = ExitStack()

    def sb(name, shape, dt):
        return es.enter_context(nc.sbuf_tensor(name, shape, dt))

    def ps(name):
        return es.enter_context(nc.psum_tensor(name, [128, 257], F32))

    def sem(name):
        return es.enter_context(nc.semaphore(name))

    zb = [sb(f"zb{i}", [128, 4096], BF16) for i in range(4)]
    zpb = [sb(f"zpb{i}", [128, 4096], BF16) for i in range(4)]
    rhs = [sb(f"rhs{i}", [128, 257], BF16) for i in range(2)]
    q = [sb(f"q{i}", [128, 256], F32) for i in range(2)]
    qp = [sb(f"qp{i}", [128, 256], F32) for i in range(2)]
    ws = sb("ws", [128, 256], F32)
    sws = sb("sws", [128, 256], F32)
    a1p = [sb(f"a1p{i}", [128, 256], F32) for i in range(2)]
    a2 = [sb(f"a2{i}", [128, 256], F32) for i in range(2)]
    a2p = [sb(f"a2p{i}", [128, 256], F32) for i in range(2)]
    cb = [sb(f"cb{i}", [128, 257], F32) for i in range(4)]
    pm = [[ps("p00"), ps("p01")], [ps("p10"), ps("p11")]]
    qz, qzp, sDV, sPE, sSC, sPO = (
        sem("qz"), sem("qzp"), sem("sDV"), sem("sPE"), sem("sSC"), sem("sPO")
    )
    sDVa, sCP, qOUT, sInit = (
        sem("sDVa"), sem("sCP"), sem("qOUT"), sem("sInit")
    )
    if True:
        NT = 16 * K  # total superchunks

        def chunk_iter():
            for it in range(K):
                for b in range(BPC):
                    for c in range(CH):
                        cg = it * 256 + b * 128 + c
                        s, cl = c // 16, c % 16
                        tg = it * 16 + b * 8 + s
                        yield it, b, c, cg, s, cl, tg

        # ---- sync (SP): z superchunk DMAs + all output DMAs ----
        nc.sync.dma_start(out=ws[:, :], in_=wsin[:]).then_inc(sInit, 16)
        nc.sync.dma_start(out=sws[:, :], in_=swin[:]).then_inc(sInit, 16)
        for tg in range(min(4, NT)):
            nc.sync.dma_start(out=zb[tg % 4][:, :], in_=zv[tg % 16]).then_inc(
                qz, 16
            )

        def emit_outputs(it, b):
            nc.sync.wait_ge(sCP, 4 * it + 2 * (b + 1))
            nc.sync.dma_start(out=cpv[2 * b], in_=cb[2 * b][:, :]).then_inc(
                qOUT, 16
            )
            nc.sync.dma_start(
                out=cpv[2 * b + 1], in_=cb[2 * b + 1][:, :]
            ).then_inc(qOUT, 16)
            nc.sync.wait_ge(sPO, it * 256 + (b + 1) * 128)
            nc.sync.wait_ge(sDVa, it * 2 + b + 1)
            nc.sync.dma_start(out=acv[3 * b], in_=a1p[b][:, :]).then_inc(
                qOUT, 16
            )
            nc.sync.dma_start(out=acv[3 * b + 1], in_=a2[b][:, :]).then_inc(
                qOUT, 16
            )
            nc.sync.dma_start(out=acv[3 * b + 2], in_=a2p[b][:, :]).then_inc(
                qOUT, 16
            )

        events = {}
        for it in range(K):
            events[16 * it + 11] = (it, 0)
            events[16 * it + 19] = (it, 1)
        for tg in range(4, NT):
            nc.sync.wait_ge(sPE, (tg - 3) * 16)
            nc.sync.wait_ge(sSC, (tg - 3) * 16)
            nc.sync.dma_start(
                out=zb[tg % 4][:, :], in_=zv[tg % 16]
            ).then_inc(qz, 16)
            if tg in events:
                emit_outputs(*events.pop(tg))
        for tg in sorted(events):
            emit_outputs(*events.pop(tg))
        nc.sync.wait_ge(qOUT, 160 * K)

        # ---- scalar (ACT): zp DMAs, squares, psum->sbuf copies ----
        for tg in range(min(4, NT)):
            nc.scalar.dma_start(
                out=zpb[tg % 4][:, :], in_=zpv[tg % 16]
            ).then_inc(qzp, 16)
        nc.scalar.wait_ge(sInit, 32)
        for it, b, c, cg, s, cl, tg in chunk_iter():
            if cl == 0:
                nc.scalar.wait_ge(qz, 16 * (tg + 1))
                nc.scalar.wait_ge(qzp, 16 * (tg + 1))
            if cg >= 2:
                nc.scalar.wait_ge(sPO, cg - 1)
            swcol = sws[:, b * 128 + c : b * 128 + c + 1]
            fs = slice(cl * 256, (cl + 1) * 256)
            nc.scalar.activation(
                q[cg % 2][:, :], zb[tg % 4][:, fs], Square, scale=swcol
            )
            nc.scalar.activation(
                qp[cg % 2][:, :], zpb[tg % 4][:, fs], Square, scale=swcol
            ).then_inc(sSC, 1)
            if cl == 15:
                tgn = tg + 4
                if tgn < NT:
                    nc.scalar.wait_ge(sDV, (tg + 1) * 16)
                    nc.scalar.dma_start(
                        out=zpb[tgn % 4][:, :], in_=zpv[tgn % 16]
                    ).then_inc(qzp, 16)
                if s == 7:  # end of batch: copy psum banks out
                    if it > 0:
                        nc.scalar.wait_ge(
                            qOUT, 16 * (10 * (it - 1) + 5 * b + 2)
                        )
                    nc.scalar.wait_ge(sPE, it * 256 + (b + 1) * 128)
                    nc.scalar.copy(
                        out=cb[2 * b][:, :], in_=pm[b][0][:, :]
                    ).then_inc(sCP, 1)
                    nc.scalar.copy(
                        out=cb[2 * b + 1][:, :], in_=pm[b][1][:, :]
                    ).then_inc(sCP, 1)

        # ---- vector (DVE): rhs = [w*zp | w], acc1p += w*zp ----
        nc.vector.wait_ge(sInit, 32)
        for it, b, c, cg, s, cl, tg in chunk_iter():
            if cl == 0:
                nc.vector.wait_ge(qzp, 16 * (tg + 1))
            if cg >= 2:
                nc.vector.wait_ge(sPE, cg - 1)
            if c == 0 and it > 0:
                nc.vector.wait_ge(qOUT, 16 * (10 * (it - 1) + 5 * b + 3))
            wcol = ws[:, b * 128 + c : b * 128 + c + 1]
            fs = slice(cl * 256, (cl + 1) * 256)
            r = rhs[cg % 2]
            nc.vector.tensor_scalar_mul(r[:, 0:256], zpb[tg % 4][:, fs], wcol)
            nc.vector.tensor_scalar_add(r[:, 256:257], wcol, 0.0).then_inc(
                sDV, 1
            )
            if c == 0:
                i3 = nc.vector.tensor_scalar_add(a1p[b][:, :], r[:, 0:256], 0.0)
            else:
                i3 = nc.vector.tensor_tensor(
                    out=a1p[b][:, :], in0=a1p[b][:, :], in1=r[:, 0:256], op=add
                )
            if c == 127:
                i3.then_inc(sDVa, 1)

        # ---- tensor (PE): 2 matmuls per chunk into psum ----
        for it, b, c, cg, s, cl, tg in chunk_iter():
            if cl == 0:
                nc.tensor.wait_ge(qz, 16 * (tg + 1))
            nc.tensor.wait_ge(sDV, cg + 1)
            if c == 0 and it > 0:
                nc.tensor.wait_ge(sCP, 4 * (it - 1) + 2 * (b + 1))
            lo = cl * 256
            st, sp = (c == 0), (c == 127)
            nc.tensor.matmul(
                pm[b][0][:, :],
                lhsT=zb[tg % 4][:, lo : lo + 128],
                rhs=rhs[cg % 2][:, :],
                start=st,
                stop=sp,
            )
            nc.tensor.matmul(
                pm[b][1][:, :],
                lhsT=zb[tg % 4][:, lo + 128 : lo + 256],
                rhs=rhs[cg % 2][:, :],
                start=st,
                stop=sp,
            ).then_inc(sPE, 1)

        # ---- gpsimd (Pool): acc2 += q, acc2p += qp ----
        for it, b, c, cg, s, cl, tg in chunk_iter():
            nc.gpsimd.wait_ge(sSC, cg + 1)
            if c == 0 and it > 0:
                nc.gpsimd.wait_ge(qOUT, 16 * (10 * (it - 1) + 5 * b + 5))
            if c == 0:
                nc.gpsimd.tensor_scalar_add(a2[b][:, :], q[cg % 2][:, :], 0.0)
                nc.gpsimd.tensor_scalar_add(
                    a2p[b][:, :], qp[cg % 2][:, :], 0.0
                ).then_inc(sPO, 1)
            else:
                nc.gpsimd.tensor_tensor(
                    out=a2[b][:, :], in0=a2[b][:, :], in1=q[cg % 2][:, :],
                    op=add,
                )
                nc.gpsimd.tensor_tensor(
                    out=a2p[b][:, :], in0=a2p[b][:, :], in1=qp[cg % 2][:, :],
                    op=add,
                ).then_inc(sPO, 1)
    es.close()
    return nc


def make_fn(iters):
    """Compile the program for 8 cores; returns fn(arrays)->(cps, accs)."""
    if iters in _CACHE:
        return _CACHE[iters]
    import jax
    from jax.sharding import Mesh, PartitionSpec
    from jax.experimental.shard_map import shard_map
    from concourse import bass2jax, mybir

    bass2jax.install_neuronx_cc_hook()
    nc = build_nc(iters)
    partition_name = (
        nc.partition_id_tensor.name if nc.partition_id_tensor else None
    )
    in_names, out_names, out_avals = [], [], []
    for alloc in nc.m.functions[0].allocations:
        if not isinstance(alloc, mybir.MemoryLocationSet):
            continue
        name = alloc.memorylocations[0].name
        if alloc.kind == "ExternalInput":
            if name != partition_name:
                in_names.append(name)
        elif alloc.kind == "ExternalOutput":
            out_names.append(name)
            out_avals.append(
                jax.core.ShapedArray(
                    tuple(alloc.tensor_shape), mybir.dt.np(alloc.dtype)
                )
            )
    n_params, n_outs = len(in_names), len(out_avals)
    all_in_names = list(in_names) + list(out_names)
    if partition_name is not None:
        all_in_names.append(partition_name)
    all_in_names = tuple(all_in_names)
    donate = tuple(range(n_params, n_params + n_outs))

    def _body(*args):
        operands = list(args)
        if partition_name is not None:
            operands.append(bass2jax.partition_id_tensor())
        outs = bass2jax._bass_exec_p.bind(
            *operands,
            out_avals=tuple(out_avals),
            in_names=all_in_names,
            out_names=tuple(out_names),
            lowering_input_output_aliases=(),
            sim_require_finite=False,
            sim_require_nnan=False,
            nc=nc,
        )
        return tuple(outs)

    devices = jax.devices()[:NCORES]
    mesh = Mesh(np.asarray(devices), ("core",))
    jitted = jax.jit(
        shard_map(
            _body,
            mesh=mesh,
            in_specs=(PartitionSpec("core"),) * (n_params + n_outs),
            out_specs=(PartitionSpec("core"),) * n_outs,
            check_rep=False,
        ),
        donate_argnums=donate,
        keep_unused=True,
    )

    def fn(arrays_by_name, device_args=None):
        import jax as _jax

        if device_args is None:
            sh = _jax.sharding.NamedSharding(mesh, PartitionSpec("core"))
            device_args = [
                _jax.device_put(arrays_by_name[n], sh) for n in in_names
            ]
        zeros = [
            np.zeros((NCORES * a.shape[0], *a.shape[1:]), a.dtype)
            for a in out_avals
        ]
        out = jitted(*device_args, *zeros)
        _jax.block_until_ready(out)
        res = {
            n: np.asarray(out[i]).reshape(
                NCORES, *out_avals[i].shape
            )
            for i, n in enumerate(out_names)
        }
        return res, device_args

    _CACHE[iters] = (fn, in_names)
    return _CACHE[iters]


def prep_inputs(z, z_prime, flat_idx):
    """Host layout: returns dict of concatenated per-core arrays."""
    w = np.zeros((B, P), np.float32)
    for b in range(B):
        w[b] = np.bincount(flat_idx[b], minlength=P)

    def lay(x):
        xc = np.ascontiguousarray(
            x.astype(BF)
            .transpose(0, 2, 3, 1)             # (B, c, p, f)
            .reshape(B, SC, Q, 128, 256)       # (B, s, q, p, f)
            .transpose(0, 1, 3, 2, 4)          # (B, s, p, q, f)
        ).reshape(B * SC, 128, 4096)
        return xc

    wr = w.reshape(B, 128, 128).transpose(0, 2, 1)  # (b, p, c)
    ws = np.ascontiguousarray(
        wr.reshape(NCORES, 2, 128, 128).transpose(0, 2, 1, 3)
    ).reshape(NCORES * 128, 256)
    return {
        "zin": lay(z),
        "zpin": lay(z_prime),
        "wsin": ws,
        "swin": np.sqrt(ws),
    }


def epilogue(cps, accs):
    """cps (8,4,128,257) f32, accs (8,6,128,256) f32 -> loss float."""
    c64 = cps.astype(np.float64)
    a64 = accs.astype(np.float64)
    top = c64[:, 0::2].sum(axis=(0, 1))   # (128,257) features 0..127
    bot = c64[:, 1::2].sum(axis=(0, 1))   # features 128..255
    Craw = np.concatenate([top[:, :256], bot[:, :256]], axis=0)
    s1 = np.concatenate([top[:, 256], bot[:, 256]])
    s1p = a64[:, 0::3].sum(axis=(0, 1, 2))
    s2 = a64[:, 1::3].sum(axis=(0, 1, 2))
    s2p = a64[:, 2::3].sum(axis=(0, 1, 2))
    mu, mup = s1 / N, s1p / N
    var = (s2 - N * mu**2) / (N - 1)
    varp = (s2p - N * mup**2) / (N - 1)
    sig = np.clip(np.sqrt(np.maximum(var, 0)), EPS, None)
    sigp = np.clip(np.sqrt(np.maximum(varp, 0)), EPS, None)
    C = (Craw - N * np.outer(mu, mup)) / (N * np.outer(sig, sigp))
    dg = np.diagonal(C)
    loss = np.sum((1.0 - dg) ** 2) + LAM * (np.sum(C**2) - np.sum(dg**2))
    return loss


def kernel(z, z_prime, flat_idx):
    z = np.asarray(z, np.float32)
    z_prime = np.asarray(z_prime, np.float32)
    flat_idx = np.asarray(flat_idx, np.int32)
    fn, _ = make_fn(1)
    arrays = prep_inputs(z, z_prime, flat_idx)
    res, _ = fn(arrays)
    cps = res["cps"].reshape(NCORES, 4, 128, 257)
    accs = res["accs"].reshape(NCORES, 6, 128, 256)
    return np.asarray(epilogue(cps, accs), dtype=np.float32)
